# revision 17
# baseline (speedup 1.0000x reference)
"""Cross-entropy loss (mean NLL) kernel for Trainium2, 8 NeuronCores.

Problem: logits [6, 16, 2, 512, 512] f32, target [16, 512, 512] int.
reference = mean over (j, b, h, w) of -log_softmax(logits, axis=2)[target].

With C=2 classes, per pixel (d = l1 - l0, t in {0, 1}):
    nll = logsumexp(l0, l1) - l_t = softplus(d) - t * d
Summing over pixels:
    sum(nll) = sum(softplus(d)) - sum(t * d)

Sharding: data-parallel over the batch axis (16 / 8 = 2 batches per core).
Each core streams its 12 (j, b) plane-pairs (2 MB each), computes per-partition
partial sums of softplus(d) and t*d via fused accumulate ops (ACT accum_out and
DVE scalar_tensor_tensor accum_out), and writes a [128, 1] partial-sum vector.
The host sums 8 x 128 partials in float64 and divides by the pixel count.
"""

import sys

import numpy as np

if "/opt/trn_rl_repo" not in sys.path:
    sys.path.insert(0, "/opt/trn_rl_repo")

import concourse.bass as bass
import concourse.tile as tile
from concourse import mybir
from concourse.bass_utils import run_bass_kernel_spmd

J, B, C, H, W = 6, 16, 2, 512, 512
NCORES = 8
BS = B // NCORES          # batches per core
P = 128                   # SBUF partitions
N = (H * W) // P          # free-dim elements per partition per plane
PAIRS = J * BS            # (j, b) plane-pairs per core
SPLIT = 2                 # halves per pair (1 DMA per half-pair)

_NC = None

# How to compute softplus(d) on the scalar engine:
#   "softplus": native Softplus op (rejected by this walrus' act tables)
#   "expln":    ln(exp(d) + 1) — 2 ACT ops, both in the single
#               natural_log_exp_and_others act set (no table switch)
ACT_MODE = "expln"


def _build(act_mode=None):
    act_mode = act_mode or ACT_MODE
    use_softplus = act_mode == "softplus"
    f32 = mybir.dt.float32
    nc = bass.Bass()
    logits = nc.dram_tensor("logits", [J, BS, C, H, W], f32, kind="ExternalInput")
    target = nc.dram_tensor("target", [BS, H, W], mybir.dt.int32, kind="ExternalInput")
    out = nc.dram_tensor("out", [P, 1], f32, kind="ExternalOutput")

    # [12, 2, 128, 2048]: per (j*b, class) plane, contiguous 8KB per partition
    planes = logits[:].rearrange("j b c (p x) w -> (j b) c p (x w)", p=P)
    tgt = target[:].rearrange("b (p x) w -> b p (x w)", p=P)

    with tile.TileContext(nc) as tc:
        with (
            tc.tile_pool(name="io", bufs=8) as io_pool,
            tc.tile_pool(name="work", bufs=3) as work_pool,
            tc.tile_pool(name="persist", bufs=1) as persist,
        ):
            # Load target once (reused across all 6 j-blocks), convert to f32.
            ti = persist.tile([P, BS * N], mybir.dt.int32)
            tf = persist.tile([P, BS * N], f32)
            for b in range(BS):
                nc.gpsimd.dma_start(out=ti[:, b * N:(b + 1) * N], in_=tgt[b])
                # int32 -> f32; one copy per DMA so each op waits on a
                # single semaphore (walrus sync-wait slot limit)
                nc.vector.tensor_copy(tf[:, b * N:(b + 1) * N], ti[:, b * N:(b + 1) * N])

            NITER = PAIRS * SPLIT
            NH = N // SPLIT  # free-dim elements per half-plane
            acc_sp = persist.tile([P, NITER], f32)
            acc_td = persist.tile([P, NITER], f32)

            for k in range(NITER):
                i, h = divmod(k, SPLIT)
                b = i % BS
                # One DMA per iteration loading both class half-planes into
                # one tile ([:, :NH] = class 0, [:, NH:] = class 1) so the
                # consumer waits on a single DMA semaphore. One DMA per
                # iteration with bufs == 8 SWDGE queues also means the slot
                # being overwritten was written by the same queue (FIFO), so
                # Tile emits no extra WAW wait (the DMA encoding only has a
                # single sync-wait slot).
                lpair = io_pool.tile([P, C * NH], f32, tag="lpair")
                nc.gpsimd.dma_start(
                    out=lpair[:].rearrange("p (c n) -> p c n", c=C),
                    in_=planes[i][:, :, h * NH:(h + 1) * NH].rearrange(
                        "c p n -> p c n"),
                )

                d = work_pool.tile([P, NH], f32, tag="d")
                nc.vector.tensor_sub(d[:], lpair[:, NH:], lpair[:, :NH])

                # td = d * t, acc_td[:, k] = sum(td) along free dim
                td = work_pool.tile([P, NH], f32, tag="td")
                nc.vector.scalar_tensor_tensor(
                    out=td[:],
                    in0=d[:],
                    scalar=1.0,
                    in1=tf[:, b * N + h * NH:b * N + (h + 1) * NH],
                    op0=mybir.AluOpType.mult,
                    op1=mybir.AluOpType.mult,
                    accum_out=acc_td[:, k:k + 1],
                )

                if use_softplus:
                    # sp = softplus(d), acc_sp[:, k] = sum(sp) along free dim
                    sp = work_pool.tile([P, NH], f32, tag="sp")
                    nc.scalar.activation(
                        sp[:],
                        d[:],
                        mybir.ActivationFunctionType.Softplus,
                        accum_out=acc_sp[:, k:k + 1],
                    )
                else:
                    # softplus(d) = ln(exp(d) + 1); both funcs live in the
                    # natural_log_exp_and_others act set, so no table switch.
                    ex = work_pool.tile([P, NH], f32, tag="ex")
                    nc.scalar.activation(
                        ex[:], d[:], mybir.ActivationFunctionType.Exp
                    )
                    lg = work_pool.tile([P, NH], f32, tag="lg")
                    nc.scalar.activation(
                        lg[:],
                        ex[:],
                        mybir.ActivationFunctionType.Ln,
                        bias=1.0,
                        accum_out=acc_sp[:, k:k + 1],
                    )

            rsp = persist.tile([P, 1], f32)
            rtd = persist.tile([P, 1], f32)
            nc.vector.reduce_sum(rsp[:], acc_sp[:], axis=mybir.AxisListType.X)
            nc.vector.reduce_sum(rtd[:], acc_td[:], axis=mybir.AxisListType.X)
            res = persist.tile([P, 1], f32)
            nc.vector.tensor_sub(res[:], rsp[:], rtd[:])
            nc.gpsimd.dma_start(out=out[:], in_=res[:])
    return _legalize_waits(nc)


def _legalize_waits(nc):
    """This walrus build only honors a single sync-wait slot per instruction
    ("Too many sync wait commands"). Tile freely attaches several. Hoist all
    but one wait onto ENGINE_NOP instructions injected immediately before the
    offender on the same engine stream — the sequencer executes wait-nops
    first, preserving semantics (for queue DMAs the issuing engine stalls
    before generating the descriptor)."""
    from concourse import mybir

    opcode = nc.isa.Opcode.NEURON_ISA_TPB_OPCODE_ENGINE_NOP
    fixable = {
        "InstTensorTensor", "InstTensorCopy", "InstTensorScalarPtr",
        "InstActivation", "InstDMACopy", "InstTensorReduce", "InstMemset",
        "InstDrain",
    }
    for f in nc.m.functions:
        for blk in f.blocks:
            out = []
            for ins in blk.instructions:
                si = ins.sync_info
                waits = list(si.on_wait or []) if si is not None else []
                is_sp = getattr(ins.engine, "name", str(ins.engine)) == "SP"
                if len(waits) > 1 and type(ins).__name__ in fixable:
                    import copy as _copy
                    for w in waits[:-1]:
                        if is_sp:
                            # SP rejects ENGINE_NOP; chain single-wait
                            # copies of the instruction itself (drain is
                            # idempotent) to carry the extra waits.
                            nop = _copy.deepcopy(ins)
                            nop.name = f"{ins.name}-w{len(out)}"
                            nop.sync_info = mybir.SyncInfo(
                                on_wait=[w], on_update=[])
                        else:
                            nop = nc.vector._isa(opcode, {}, None, [], [], True)
                            nop.engine = ins.engine
                            nop.sync_info = mybir.SyncInfo(on_wait=[w], on_update=[])
                        out.append(nop)
                    ins.sync_info = mybir.SyncInfo(
                        on_wait=[waits[-1]], on_update=list(si.on_update or [])
                    )
                out.append(ins)
            blk.instructions[:] = out
    return nc


def _in_maps(logits, target_i32):
    maps = []
    for c in range(NCORES):
        b0 = c * BS
        maps.append({
            "logits": np.ascontiguousarray(logits[:, b0:b0 + BS]),
            "target": np.ascontiguousarray(target_i32[b0:b0 + BS]),
        })
    return maps


def _run(logits, target, **kw):
    """Returns (mean_loss, BassKernelResults)."""
    global _NC
    logits = np.asarray(logits, dtype=np.float32)
    target_i32 = np.asarray(target).astype(np.int32)
    assert logits.shape == (J, B, C, H, W), logits.shape
    assert target_i32.shape == (B, H, W), target_i32.shape
    if _NC is None:
        _NC = _build()
    r = run_bass_kernel_spmd(_NC, _in_maps(logits, target_i32), list(range(NCORES)), **kw)
    total = 0.0
    for res in r.results:
        total += float(np.sum(res["out"].astype(np.float64)))
    mean = np.float32(total / (J * B * H * W))
    return mean, r


def kernel(logits, target):
    mean, _ = _run(logits, target)
    return mean
